# revision 8
# baseline (speedup 1.0000x reference)
"""Bahdanau-style attention kernel for Trainium2, 8-core data parallel.

reference math (per batch b):
  w1 = q @ W1 + b1                       [B, A]
  w2 = hs @ W2 + b2                      [B, T, A]
  score = tanh(w1[:, None, :] + w2)      [B, T, A]
  logits = score @ V + bV               [B, T, 1]
  attn = softmax(logits over T)
  ctx = sum_T attn * hs                  [B, F]
returns (ctx, attn)

Sharding: pure data parallel over batch; each of 8 cores gets 128 batches.

Per-core plan (fp32 storage, fp32r matmuls at 1 cycle/row):
  hs arrives [bt, f] row-major; PE contracts over partitions, so hs is
  transposed on-chip via PE transpose mode into [f, bt] tiles, then
  w2^T[a, bt] = W2_chunk^T @ hs^T accumulated over f-chunks in PSUM.
  score = tanh(w2^T + w1^T broadcast over t), logits via V^T @ score,
  softmax over t done in the free dim (exp row, segmented sum, scale),
  attention columns recovered with K=1 matmuls, context accumulated in
  PSUM via block-diagonal selection matmuls against the natural-layout hs.
"""

import numpy as np

B, T, F, G, A = 1024, 64, 2048, 1024, 512
NCORES = 8
BC = B // NCORES              # 128 batches per core
ROWS = BC * T                 # 8192 bt rows per core
NT = ROWS // 128              # 64 tiles of 128 rows (2 batches each)
NSUP = NT // 4                # 16 supertiles of 512 rows (8 batches each)
FCN = F // 128                # 16 f-chunks
ACN = A // 128                # 4 a-chunks
GCN = G // 128                # 8 g-chunks

_CACHE = {}


def _build_nc():
    import concourse.bass as bass
    import concourse.mybir as mybir
    import concourse.tile as tile
    from concourse import bacc
    from concourse.masks import make_identity

    F32 = mybir.dt.float32
    F32R = mybir.dt.float32r

    nc = bacc.Bacc("TRN2", target_bir_lowering=False, debug=False, num_devices=NCORES)

    q_d = nc.dram_tensor("q", [BC, G], F32R, kind="ExternalInput").ap()
    hs_d = nc.dram_tensor("hs", [ROWS, F], F32R, kind="ExternalInput").ap()
    w1_d = nc.dram_tensor("W1", [G, A], F32R, kind="ExternalInput").ap()
    w2_d = nc.dram_tensor("W2", [F, A], F32R, kind="ExternalInput").ap()
    v_d = nc.dram_tensor("Vc", [A], F32R, kind="ExternalInput").ap()
    bias_d = nc.dram_tensor("biascol", [128, ACN], F32, kind="ExternalInput").ap()
    ctx_d = nc.dram_tensor("ctx", [BC, F], F32, kind="ExternalOutput").ap()
    attn_d = nc.dram_tensor("attn", [ROWS], F32R, kind="ExternalOutput").ap()

    with tile.TileContext(nc) as tc:
        with tc.tile_pool(name="persist", bufs=1) as persist:
            # --- persistent constants ---
            w2_sb = persist.tile([128, FCN, A], F32R)       # W2 chunks [f%, fc, a]
            nc.sync.dma_start(out=w2_sb, in_=w2_d.rearrange("(fc p) a -> p fc a", p=128))
            v_sb = persist.tile([128, ACN], F32R)           # V [a%, ac]
            nc.sync.dma_start(out=v_sb, in_=v_d.rearrange("(c p) -> p c", p=128))
            bias_sb = persist.tile([128, ACN], F32)         # (b1+b2) [a%, ac]
            nc.sync.dma_start(out=bias_sb, in_=bias_d)
            ident_f32 = persist.tile([128, 128], F32)
            make_identity(nc, ident_f32)
            ident = persist.tile([128, 128], F32R)
            nc.vector.tensor_copy(ident, ident_f32)
            one_sb = persist.tile([1, 1], F32)
            nc.vector.memset(one_sb, 1.0)
            zeros32_f32 = persist.tile([128, 32], F32)
            nc.vector.memset(zeros32_f32, 0.0)
            zeros32 = persist.tile([128, 32], F32R)
            nc.vector.tensor_copy(zeros32, zeros32_f32)
            w1t_sb = persist.tile([128, ACN, BC], F32)      # w1^T [a%, ac, b]

            # --- preamble: w1^T = (q @ W1 + b1 + b2)^T  -> [a, b] layout ---
            with (
                tc.tile_pool(name="pre", bufs=1) as pre,
                tc.tile_pool(name="prep", bufs=2, space="PSUM") as prep,
            ):
                q_sb = pre.tile([128, G], F32R)
                nc.sync.dma_start(out=q_sb, in_=q_d[:, :])
                w1w_sb = pre.tile([128, GCN, A], F32R)      # W1 chunks [g%, gc, a]
                nc.sync.dma_start(out=w1w_sb, in_=w1_d.rearrange("(gc p) a -> p gc a", p=128))
                qt_sb = pre.tile([128, GCN, 128], F32R)     # q^T [g%, gc, b]
                for gc in range(GCN):
                    tp = prep.tile([128, 128], F32R, tag="pp")
                    nc.tensor.transpose(tp, q_sb[:, 128 * gc:128 * (gc + 1)], ident)
                    nc.vector.tensor_copy(qt_sb[:, gc, :], tp)
                for ac in range(ACN):
                    wp = prep.tile([128, BC], F32, tag="pp")
                    for gc in range(GCN):
                        nc.tensor.matmul(
                            wp,
                            w1w_sb[:, gc, 128 * ac:128 * (ac + 1)],
                            qt_sb[:, gc, :],
                            start=(gc == 0), stop=(gc == GCN - 1),
                        )
                    nc.vector.tensor_scalar_add(w1t_sb[:, ac, :], wp, bias_sb[:, ac:ac + 1])

            # --- main pipeline ---
            with (
                tc.tile_pool(name="hp", bufs=8) as hp,
                tc.tile_pool(name="hstp", bufs=1) as hstp,
                tc.tile_pool(name="scorep", bufs=2) as scorep,
                tc.tile_pool(name="rowsp", bufs=4) as rowsp,
                tc.tile_pool(name="selp", bufs=3) as selp,
                tc.tile_pool(name="outp", bufs=1) as outp,
                tc.tile_pool(name="ctxpp", bufs=1, space="PSUM") as ctxpp,
                tc.tile_pool(name="w2pp", bufs=2, space="PSUM") as w2pp,
                tc.tile_pool(name="scrp", bufs=2, space="PSUM") as scrp,
            ):
                ctxps = ctxpp.tile([32, F], F32)
                ctx_sb = outp.tile([128, F], F32)

                for s in range(NSUP):
                    hts = []
                    hst = hstp.tile([128, FCN, 512], F32R)
                    for j in range(4):
                        i = 4 * s + j
                        ht = hp.tile([128, F], F32R, tag="h")
                        nc.sync.dma_start(out=ht, in_=hs_d[128 * i:128 * (i + 1), :])
                        hts.append(ht)
                        for fc in range(FCN):
                            tp = scrp.tile([128, 128], F32R, tag="scr")
                            nc.tensor.transpose(tp, ht[:, 128 * fc:128 * (fc + 1)], ident)
                            nc.any.tensor_copy(hst[:, fc, 128 * j:128 * (j + 1)], tp)

                    # w2^T = W2c^T @ hs^T per a-chunk; + w1 bcast; tanh
                    score = scorep.tile([128, ACN, 512], F32R)
                    for ac in range(ACN):
                        w2ps = w2pp.tile([128, 512], F32, tag="w2")
                        for fc in range(FCN):
                            nc.tensor.matmul(
                                w2ps,
                                w2_sb[:, fc, 128 * ac:128 * (ac + 1)],
                                hst[:, fc, :],
                                start=(fc == 0), stop=(fc == FCN - 1),
                            )
                        w1b = bass.AP(
                            tensor=w1t_sb.tensor,
                            offset=w1t_sb.offset + ac * BC + 8 * s,
                            ap=[list(w1t_sb.ap[0]), [1, 8], [0, 64]],
                        )
                        nc.vector.tensor_add(
                            score[:, ac, :].rearrange("p (b t) -> p b t", b=8),
                            w2ps.rearrange("p (b t) -> p b t", b=8),
                            w1b,
                        )
                        nc.scalar.activation(
                            score[:, ac, :], score[:, ac, :],
                            mybir.ActivationFunctionType.Tanh,
                        )

                    # logits row [1, 512] via V^T @ score
                    lps = scrp.tile([1, 512], F32, tag="scr")
                    for ac in range(ACN):
                        nc.tensor.matmul(
                            lps, v_sb[:, ac:ac + 1], score[:, ac, :],
                            start=(ac == 0), stop=(ac == ACN - 1),
                        )
                    # softmax over t in the free dim (no max subtraction:
                    # |logits| <= sum|V| stays far from fp32 overflow)
                    erow = rowsp.tile([1, 512], F32, tag="row")
                    nc.scalar.activation(erow, lps, mybir.ActivationFunctionType.Exp)
                    srow = rowsp.tile([1, 8], F32, tag="s8")
                    nc.vector.reduce_sum(
                        srow, erow.rearrange("p (b t) -> p b t", b=8),
                        axis=mybir.AxisListType.X,
                    )
                    rrow = rowsp.tile([1, 8], F32, tag="s8")
                    nc.vector.reciprocal(rrow, srow)
                    arow = rowsp.tile([1, 512], F32R, tag="row")
                    rb = bass.AP(tensor=rrow.tensor, offset=rrow.offset,
                                 ap=[list(rrow.ap[0]), [1, 8], [0, 64]])
                    nc.vector.tensor_mul(
                        arow.rearrange("p (b t) -> p b t", b=8),
                        erow.rearrange("p (b t) -> p b t", b=8),
                        rb,
                    )
                    nc.sync.dma_start(out=attn_d[512 * s:512 * (s + 1)], in_=arow)

                    # attention columns + context accumulation
                    for j in range(4):
                        i = 4 * s + j
                        g, k = divmod(i, 16)
                        cps = scrp.tile([128, 1], F32, tag="scr")
                        nc.tensor.matmul(
                            cps, arow[0:1, 128 * j:128 * (j + 1)].bitcast(F32), one_sb,
                            start=True, stop=True,
                        )
                        sel = selp.tile([128, 32], F32R, tag="sel")
                        nc.vector.tensor_copy(sel, zeros32)
                        nc.vector.tensor_copy(sel[0:64, 2 * k:2 * k + 1], cps[0:64, :])
                        nc.vector.tensor_copy(sel[64:128, 2 * k + 1:2 * k + 2], cps[64:128, :])
                        for c in range(4):
                            nc.tensor.matmul(
                                ctxps[:, 512 * c:512 * (c + 1)],
                                sel, hts[j][:, 512 * c:512 * (c + 1)],
                                start=(k == 0), stop=(k == 15),
                            )
                        if k == 15:
                            for c in range(4):
                                nc.vector.tensor_copy(
                                    ctx_sb[32 * g:32 * (g + 1), 512 * c:512 * (c + 1)],
                                    ctxps[:, 512 * c:512 * (c + 1)],
                                )

                nc.sync.dma_start(out=ctx_d[:, :], in_=ctx_sb)

    nc.compile()
    return nc


def _get_runner():
    """Build + compile once per process; returns fn(in_maps) -> per-core outs."""
    if "runner" in _CACHE:
        return _CACHE["runner"]

    import jax
    import jax.numpy as jnp
    import concourse.mybir as mybir
    from jax.sharding import Mesh, PartitionSpec
    from jax.experimental.shard_map import shard_map
    from concourse.bass2jax import _bass_exec_p, install_neuronx_cc_hook, partition_id_tensor
    from concourse.bass_interp import get_hw_module

    nc = _build_nc()
    nc.m = get_hw_module(nc.m)
    install_neuronx_cc_hook()

    partition_name = nc.partition_id_tensor.name if nc.partition_id_tensor else None
    in_names, out_names, out_avals, zero_shapes = [], [], [], []
    for alloc in nc.m.functions[0].allocations:
        if not isinstance(alloc, mybir.MemoryLocationSet):
            continue
        name = alloc.memorylocations[0].name
        if alloc.kind == "ExternalInput":
            if name != partition_name:
                in_names.append(name)
        elif alloc.kind == "ExternalOutput":
            shape = tuple(alloc.tensor_shape)
            dtype = mybir.dt.np(alloc.dtype)
            out_names.append(name)
            out_avals.append(jax.core.ShapedArray(shape, dtype))
            zero_shapes.append((shape, dtype))
    n_params = len(in_names)
    all_in_names = in_names + out_names
    if partition_name is not None:
        all_in_names = all_in_names + [partition_name]

    def _body(*args):
        operands = list(args)
        if partition_name is not None:
            operands.append(partition_id_tensor())
        outs = _bass_exec_p.bind(
            *operands,
            out_avals=tuple(out_avals),
            in_names=tuple(all_in_names),
            out_names=tuple(out_names),
            lowering_input_output_aliases=(),
            sim_require_finite=True,
            sim_require_nnan=True,
            nc=nc,
        )
        return tuple(outs)

    devices = jax.devices()[:NCORES]
    mesh = Mesh(np.asarray(devices), ("core",))
    n_outs = len(out_names)
    sharded = jax.jit(
        shard_map(
            _body, mesh=mesh,
            in_specs=(PartitionSpec("core"),) * (n_params + n_outs),
            out_specs=(PartitionSpec("core"),) * n_outs,
            check_rep=False,
        ),
        donate_argnums=tuple(range(n_params, n_params + n_outs)),
        keep_unused=True,
    )

    def run(in_maps):
        concat_in = [
            np.concatenate([np.asarray(m[name]) for m in in_maps], axis=0)
            for name in in_names
        ]
        concat_zeros = [
            jnp.zeros((NCORES * shp[0], *shp[1:]), dt) for shp, dt in zero_shapes
        ]
        out_arrs = sharded(*concat_in, *concat_zeros)
        return [
            {
                name: np.asarray(out_arrs[oi]).reshape(NCORES, *out_avals[oi].shape)[c]
                for oi, name in enumerate(out_names)
            }
            for c in range(NCORES)
        ]

    _CACHE["runner"] = run
    return run


def _make_in_maps(query, hidden_states, W1, b1, W2, b2, V, bV):
    query = np.asarray(query, dtype=np.float32)
    hidden_states = np.asarray(hidden_states, dtype=np.float32)
    bias = np.asarray(b1, np.float32) + np.asarray(b2, np.float32)
    bias_col = bias.reshape(ACN, 128).T.copy()  # [128, ACN]: bias_col[p, c] = bias[128c+p]
    w1 = np.ascontiguousarray(np.asarray(W1, np.float32))
    w2 = np.ascontiguousarray(np.asarray(W2, np.float32))
    v = np.ascontiguousarray(np.asarray(V, np.float32).reshape(A))
    in_maps = []
    for c in range(NCORES):
        sl = slice(c * BC, (c + 1) * BC)
        in_maps.append({
            "q": np.ascontiguousarray(query[sl]),
            "hs": np.ascontiguousarray(hidden_states[sl].reshape(ROWS, F)),
            "W1": w1,
            "W2": w2,
            "Vc": v,
            "biascol": bias_col,
        })
    return in_maps


def kernel(query, hidden_states, W1, b1, W2, b2, V, bV):
    run = _get_runner()
    in_maps = _make_in_maps(query, hidden_states, W1, b1, W2, b2, V, bV)
    res = run(in_maps)
    ctx = np.concatenate([res[c]["ctx"] for c in range(NCORES)], axis=0)
    attn = np.concatenate(
        [res[c]["attn"].reshape(BC, T, 1) for c in range(NCORES)], axis=0
    )
    return ctx.astype(np.float32), attn.astype(np.float32)


# revision 9
# speedup vs baseline: 192.1041x; 192.1041x over previous
"""Bahdanau-style attention kernel for Trainium2, 8-core data parallel.

reference math (per batch b):
  w1 = q @ W1 + b1                       [B, A]
  w2 = hs @ W2 + b2                      [B, T, A]
  score = tanh(w1[:, None, :] + w2)      [B, T, A]
  logits = score @ V + bV               [B, T, 1]
  attn = softmax(logits over T)
  ctx = sum_T attn * hs                  [B, F]
returns (ctx, attn)

Sharding: pure data parallel over batch; each of 8 cores gets 128 batches.

Per-core plan (fp32 storage, fp32r matmuls at 1 cycle/row):
  hs arrives [bt, f] row-major; PE contracts over partitions, so hs is
  transposed on-chip via PE transpose mode into [f, bt] tiles, then
  w2^T[a, bt] = W2_chunk^T @ hs^T accumulated over f-chunks in PSUM.
  score = tanh(w2^T + w1^T broadcast over t), logits via V^T @ score,
  softmax over t done in the free dim (exp row, segmented sum, scale),
  attention columns recovered with K=1 matmuls, context accumulated in
  PSUM via block-diagonal selection matmuls against the natural-layout hs.
"""

import numpy as np

B, T, F, G, A = 1024, 64, 2048, 1024, 512
NCORES = 8
BC = B // NCORES              # 128 batches per core
ROWS = BC * T                 # 8192 bt rows per core
NT = ROWS // 128              # 64 tiles of 128 rows (2 batches each)
NSUP = NT // 4                # 16 supertiles of 512 rows (8 batches each)
FCN = F // 128                # 16 f-chunks
ACN = A // 128                # 4 a-chunks
GCN = G // 128                # 8 g-chunks

_CACHE = {}


def _build_nc():
    import concourse.bass as bass
    import concourse.mybir as mybir
    import concourse.tile as tile
    from concourse import bacc
    from concourse.masks import make_identity

    F32 = mybir.dt.float32
    F32R = mybir.dt.float32r

    nc = bacc.Bacc("TRN2", target_bir_lowering=False, debug=False, num_devices=NCORES)

    q_d = nc.dram_tensor("q", [BC, G], F32R, kind="ExternalInput").ap()
    hs_d = nc.dram_tensor("hs", [ROWS, F], F32R, kind="ExternalInput").ap()
    w1_d = nc.dram_tensor("W1", [G, A], F32R, kind="ExternalInput").ap()
    w2_d = nc.dram_tensor("W2", [F, A], F32R, kind="ExternalInput").ap()
    v_d = nc.dram_tensor("Vc", [A], F32R, kind="ExternalInput").ap()
    bias_d = nc.dram_tensor("biascol", [128, ACN], F32, kind="ExternalInput").ap()
    ctx_d = nc.dram_tensor("ctx", [BC, F], F32, kind="ExternalOutput").ap()
    attn_d = nc.dram_tensor("attn", [ROWS], F32R, kind="ExternalOutput").ap()

    with tile.TileContext(nc) as tc:
        with tc.tile_pool(name="persist", bufs=1) as persist:
            # --- persistent constants ---
            w2_sb = persist.tile([128, FCN, A], F32R)       # W2 chunks [f%, fc, a]
            nc.sync.dma_start(out=w2_sb, in_=w2_d.rearrange("(fc p) a -> p fc a", p=128))
            v_sb = persist.tile([128, ACN], F32R)           # V [a%, ac]
            nc.sync.dma_start(out=v_sb, in_=v_d.rearrange("(c p) -> p c", p=128))
            bias_sb = persist.tile([128, ACN], F32)         # (b1+b2) [a%, ac]
            nc.sync.dma_start(out=bias_sb, in_=bias_d)
            ident_f32 = persist.tile([128, 128], F32)
            make_identity(nc, ident_f32)
            ident = persist.tile([128, 128], F32R)
            nc.vector.tensor_copy(ident, ident_f32)
            one_sb = persist.tile([1, 1], F32)
            nc.vector.memset(one_sb, 1.0)
            zeros32_f32 = persist.tile([128, 32], F32)
            nc.vector.memset(zeros32_f32, 0.0)
            zeros32 = persist.tile([128, 32], F32R)
            nc.vector.tensor_copy(zeros32, zeros32_f32)
            w1t_sb = persist.tile([128, ACN, BC], F32)      # w1^T [a%, ac, b]

            # --- preamble: w1^T = (q @ W1 + b1 + b2)^T  -> [a, b] layout ---
            with (
                tc.tile_pool(name="pre", bufs=1) as pre,
                tc.tile_pool(name="prep", bufs=2, space="PSUM") as prep,
            ):
                q_sb = pre.tile([128, G], F32R)
                nc.sync.dma_start(out=q_sb, in_=q_d[:, :])
                w1w_sb = pre.tile([128, GCN, A], F32R)      # W1 chunks [g%, gc, a]
                nc.sync.dma_start(out=w1w_sb, in_=w1_d.rearrange("(gc p) a -> p gc a", p=128))
                qt_sb = pre.tile([128, GCN, 128], F32R)     # q^T [g%, gc, b]
                for gc in range(GCN):
                    tp = prep.tile([128, 128], F32R, tag="pp")
                    nc.tensor.transpose(tp, q_sb[:, 128 * gc:128 * (gc + 1)], ident)
                    nc.vector.tensor_copy(qt_sb[:, gc, :], tp)
                for ac in range(ACN):
                    wp = prep.tile([128, BC], F32, tag="pp")
                    for gc in range(GCN):
                        nc.tensor.matmul(
                            wp,
                            w1w_sb[:, gc, 128 * ac:128 * (ac + 1)],
                            qt_sb[:, gc, :],
                            start=(gc == 0), stop=(gc == GCN - 1),
                        )
                    nc.vector.tensor_scalar_add(w1t_sb[:, ac, :], wp, bias_sb[:, ac:ac + 1])

            # --- main pipeline ---
            with (
                tc.tile_pool(name="hp", bufs=8) as hp,
                tc.tile_pool(name="hstp", bufs=1) as hstp,
                tc.tile_pool(name="scorep", bufs=2) as scorep,
                tc.tile_pool(name="rowsp", bufs=4) as rowsp,
                tc.tile_pool(name="selp", bufs=3) as selp,
                tc.tile_pool(name="outp", bufs=1) as outp,
                tc.tile_pool(name="ctxpp", bufs=1, space="PSUM") as ctxpp,
                tc.tile_pool(name="w2pp", bufs=2, space="PSUM") as w2pp,
                tc.tile_pool(name="scrp", bufs=2, space="PSUM") as scrp,
            ):
                ctxps = ctxpp.tile([32, F], F32)
                ctx_sb = outp.tile([128, F], F32)

                for s in range(NSUP):
                    hts = []
                    hst = hstp.tile([128, FCN, 512], F32R)
                    for j in range(4):
                        i = 4 * s + j
                        ht = hp.tile([128, F], F32R, tag="h")
                        nc.sync.dma_start(out=ht, in_=hs_d[128 * i:128 * (i + 1), :])
                        hts.append(ht)
                        for fc in range(FCN):
                            tp = scrp.tile([128, 128], F32R, tag="scr")
                            nc.tensor.transpose(tp, ht[:, 128 * fc:128 * (fc + 1)], ident)
                            nc.any.tensor_copy(hst[:, fc, 128 * j:128 * (j + 1)], tp)

                    # w2^T = W2c^T @ hs^T per a-chunk; + w1 bcast; tanh
                    score = scorep.tile([128, ACN, 512], F32R)
                    for ac in range(ACN):
                        w2ps = w2pp.tile([128, 512], F32, tag="w2")
                        for fc in range(FCN):
                            nc.tensor.matmul(
                                w2ps,
                                w2_sb[:, fc, 128 * ac:128 * (ac + 1)],
                                hst[:, fc, :],
                                start=(fc == 0), stop=(fc == FCN - 1),
                            )
                        w1b = bass.AP(
                            tensor=w1t_sb.tensor,
                            offset=w1t_sb.offset + ac * BC + 8 * s,
                            ap=[list(w1t_sb.ap[0]), [1, 8], [0, 64]],
                        )
                        nc.vector.tensor_add(
                            score[:, ac, :].rearrange("p (b t) -> p b t", b=8),
                            w2ps.rearrange("p (b t) -> p b t", b=8),
                            w1b,
                        )
                        nc.scalar.activation(
                            score[:, ac, :], score[:, ac, :],
                            mybir.ActivationFunctionType.Tanh,
                        )

                    # logits row [1, 512] via V^T @ score
                    lps = scrp.tile([1, 512], F32, tag="scr")
                    for ac in range(ACN):
                        nc.tensor.matmul(
                            lps, v_sb[:, ac:ac + 1], score[:, ac, :],
                            start=(ac == 0), stop=(ac == ACN - 1),
                        )
                    # softmax over t in the free dim (no max subtraction:
                    # |logits| <= sum|V| stays far from fp32 overflow)
                    erow = rowsp.tile([1, 512], F32, tag="row")
                    nc.scalar.activation(erow, lps, mybir.ActivationFunctionType.Exp)
                    srow = rowsp.tile([1, 8], F32, tag="s8")
                    nc.vector.reduce_sum(
                        srow, erow.rearrange("p (b t) -> p b t", b=8),
                        axis=mybir.AxisListType.X,
                    )
                    rrow = rowsp.tile([1, 8], F32, tag="s8")
                    nc.vector.reciprocal(rrow, srow)
                    arow = rowsp.tile([1, 512], F32R, tag="row")
                    rb = bass.AP(tensor=rrow.tensor, offset=rrow.offset,
                                 ap=[list(rrow.ap[0]), [1, 8], [0, 64]])
                    nc.vector.tensor_mul(
                        arow.rearrange("p (b t) -> p b t", b=8),
                        erow.rearrange("p (b t) -> p b t", b=8),
                        rb,
                    )
                    nc.sync.dma_start(out=attn_d[512 * s:512 * (s + 1)], in_=arow)

                    # attention columns + context accumulation
                    for j in range(4):
                        i = 4 * s + j
                        g, k = divmod(i, 16)
                        cps = scrp.tile([128, 1], F32, tag="scr")
                        nc.tensor.matmul(
                            cps, arow[0:1, 128 * j:128 * (j + 1)].bitcast(F32), one_sb,
                            start=True, stop=True,
                        )
                        sel = selp.tile([128, 32], F32R, tag="sel")
                        nc.vector.tensor_copy(sel, zeros32)
                        nc.vector.tensor_copy(sel[0:64, 2 * k:2 * k + 1], cps[0:64, :])
                        nc.vector.tensor_copy(sel[64:128, 2 * k + 1:2 * k + 2], cps[64:128, :])
                        for c in range(4):
                            nc.tensor.matmul(
                                ctxps[:, 512 * c:512 * (c + 1)],
                                sel, hts[j][:, 512 * c:512 * (c + 1)],
                                start=(k == 0), stop=(k == 15),
                            )
                        if k == 15:
                            for c in range(4):
                                nc.vector.tensor_copy(
                                    ctx_sb[32 * g:32 * (g + 1), 512 * c:512 * (c + 1)],
                                    ctxps[:, 512 * c:512 * (c + 1)],
                                )

                nc.sync.dma_start(out=ctx_d[:, :], in_=ctx_sb)

    nc.compile()
    return nc


def _get_runner():
    """Build + compile once per process; returns fn(in_maps) -> per-core outs."""
    if "runner" in _CACHE:
        return _CACHE["runner"]

    import jax
    import jax.numpy as jnp
    import concourse.mybir as mybir
    from jax.sharding import Mesh, PartitionSpec
    from jax.experimental.shard_map import shard_map
    from concourse.bass2jax import _bass_exec_p, install_neuronx_cc_hook, partition_id_tensor
    from concourse.bass_interp import get_hw_module

    nc = _build_nc()
    nc.m = get_hw_module(nc.m)
    install_neuronx_cc_hook()

    partition_name = nc.partition_id_tensor.name if nc.partition_id_tensor else None
    in_names, out_names, out_avals, zero_shapes = [], [], [], []
    for alloc in nc.m.functions[0].allocations:
        if not isinstance(alloc, mybir.MemoryLocationSet):
            continue
        name = alloc.memorylocations[0].name
        if alloc.kind == "ExternalInput":
            if name != partition_name:
                in_names.append(name)
        elif alloc.kind == "ExternalOutput":
            shape = tuple(alloc.tensor_shape)
            dtype = mybir.dt.np(alloc.dtype)
            out_names.append(name)
            out_avals.append(jax.core.ShapedArray(shape, dtype))
            zero_shapes.append((shape, dtype))
    n_params = len(in_names)
    all_in_names = in_names + out_names
    if partition_name is not None:
        all_in_names = all_in_names + [partition_name]

    def _body(*args):
        operands = list(args)
        if partition_name is not None:
            operands.append(partition_id_tensor())
        outs = _bass_exec_p.bind(
            *operands,
            out_avals=tuple(out_avals),
            in_names=tuple(all_in_names),
            out_names=tuple(out_names),
            lowering_input_output_aliases=(),
            sim_require_finite=True,
            sim_require_nnan=True,
            nc=nc,
        )
        return tuple(outs)

    from jax.sharding import NamedSharding

    devices = jax.devices()[:NCORES]
    mesh = Mesh(np.asarray(devices), ("core",))
    core_sharding = NamedSharding(mesh, PartitionSpec("core"))
    n_outs = len(out_names)
    sharded = jax.jit(
        shard_map(
            _body, mesh=mesh,
            in_specs=(PartitionSpec("core"),) * (n_params + n_outs),
            out_specs=(PartitionSpec("core"),) * n_outs,
            check_rep=False,
        ),
        donate_argnums=tuple(range(n_params, n_params + n_outs)),
        keep_unused=True,
    )
    zeros_fn = jax.jit(
        lambda: tuple(
            jnp.zeros((NCORES * shp[0], *shp[1:]), dt) for shp, dt in zero_shapes
        ),
        out_shardings=tuple(core_sharding for _ in zero_shapes),
    )

    def prepare(in_maps):
        concat_in = [
            np.concatenate([np.asarray(m[name]) for m in in_maps], axis=0)
            for name in in_names
        ]
        return [jax.device_put(a, core_sharding) for a in concat_in]

    def execute(dev_in, fetch=True):
        out_arrs = sharded(*dev_in, *zeros_fn())
        if not fetch:
            jax.block_until_ready(out_arrs)
            return None
        return [
            {
                name: np.asarray(out_arrs[oi]).reshape(NCORES, *out_avals[oi].shape)[c]
                for oi, name in enumerate(out_names)
            }
            for c in range(NCORES)
        ]

    def run(in_maps):
        return execute(prepare(in_maps))

    run.prepare = prepare
    run.execute = execute
    _CACHE["runner"] = run
    return run


def _make_in_maps(query, hidden_states, W1, b1, W2, b2, V, bV):
    query = np.asarray(query, dtype=np.float32)
    hidden_states = np.asarray(hidden_states, dtype=np.float32)
    bias = np.asarray(b1, np.float32) + np.asarray(b2, np.float32)
    bias_col = bias.reshape(ACN, 128).T.copy()  # [128, ACN]: bias_col[p, c] = bias[128c+p]
    w1 = np.ascontiguousarray(np.asarray(W1, np.float32))
    w2 = np.ascontiguousarray(np.asarray(W2, np.float32))
    v = np.ascontiguousarray(np.asarray(V, np.float32).reshape(A))
    in_maps = []
    for c in range(NCORES):
        sl = slice(c * BC, (c + 1) * BC)
        in_maps.append({
            "q": np.ascontiguousarray(query[sl]),
            "hs": np.ascontiguousarray(hidden_states[sl].reshape(ROWS, F)),
            "W1": w1,
            "W2": w2,
            "Vc": v,
            "biascol": bias_col,
        })
    return in_maps


def kernel(query, hidden_states, W1, b1, W2, b2, V, bV):
    run = _get_runner()
    in_maps = _make_in_maps(query, hidden_states, W1, b1, W2, b2, V, bV)
    res = run(in_maps)
    ctx = np.concatenate([res[c]["ctx"] for c in range(NCORES)], axis=0)
    attn = np.concatenate(
        [res[c]["attn"].reshape(BC, T, 1) for c in range(NCORES)], axis=0
    )
    return ctx.astype(np.float32), attn.astype(np.float32)
